# revision 1
# baseline (speedup 1.0000x reference)
"""NextVLAD Trainium2 kernel — 8-way data-parallel over batch (1 sample/core).

Per-core dataflow (sample b, M=512 tokens, N=1024 feat, E*N=2048, G=8, K=128, D=256):
  X [n,m]      <- host-reordered input slice (n on partitions, tokens m on free)
  inv[*,m]     = 1/sqrt(sum_n X^2)   via ones-matmul partition reduction (broadcast rows)
  y[e,m]       = (W_inp.T-chunks)^T X * inv            (no bias: b_inp cancels in softmax)
  yT[m,e]      = PE-transpose(y) + b_inp               (einsum operand, group-interleaved
                                                        layout with ones columns)
  sg[g,m]      = sigmoid((W_g@W_inp) X * inv + b_g')   b_g' = b_g + W_g@b_inp (host)
  logits[m,gk] = y^T W_gk.T-chunks ; ex = exp(logits)  (softmax max-sub skipped; b_gk cancels)
  ise          = 1/sum_m ex          via ones-matmul
  wf[m,gk]     = ex * sg * ise
  vlad[k,d+2]  = sum_{g,m} wf^T [yT | 1 1]  (ones cols give S[k] = sum w in col D)
  out[k,d]     = l2norm_d(vlad - S*cent) / sqrt(128)   (global l2norm == /sqrt(128))

Big matmuls run as float32r (full PE rate at even free-dim>=256).
"""
import os
import numpy as np

N = 1024          # feature size
EN = 2048         # expanded features
G = 8             # groups
KC = 128          # clusters
D = 256           # per-group cluster dim
BW = D + 2        # group block width in yT (data + ones + pad)
M = 512           # tokens per sample (8*8*8)
NT = N // 128     # 8 contraction chunks over n
ET = EN // 128    # 16 e-tiles
MT = 4            # m-tiles of 128
GH = 2            # gk halves of 512

_cache = {}


def _build_nc():
    import concourse.bacc as bacc
    import concourse.tile as tile
    from concourse import mybir

    f32 = mybir.dt.float32
    f32r = mybir.dt.float32r
    bf16 = mybir.dt.bfloat16
    fp8 = mybir.dt.float8e4
    Alu = mybir.AluOpType
    Act = mybir.ActivationFunctionType

    nc = bacc.Bacc("TRN2", target_bir_lowering=False)
    x_d = nc.dram_tensor("x", [N, M], fp8, kind="ExternalInput")
    w1_d = nc.dram_tensor("w1", [N, EN + G], fp8, kind="ExternalInput")
    w2_d = nc.dram_tensor("w2", [EN, G * KC], fp8, kind="ExternalInput")
    binp_d = nc.dram_tensor("binp", [1, EN], f32, kind="ExternalInput")
    cr_d = nc.dram_tensor("cr", [128, 256], f32r, kind="ExternalInput")   # ones|identr
    crb_d = nc.dram_tensor("crb", [128, 128], bf16, kind="ExternalInput")  # bf16 ones
    cf_d = nc.dram_tensor("cf", [128, D + 10], f32, kind="ExternalInput") # centn|eye8|bg
    out_d = nc.dram_tensor("out", [KC, D], f32, kind="ExternalOutput")

    with tile.TileContext(nc) as tc:
        with tc.tile_pool(name="const", bufs=1) as constp, \
             tc.tile_pool(name="persist", bufs=1) as persist, \
             tc.tile_pool(name="w2p", bufs=3) as w2p, \
             tc.tile_pool(name="fin", bufs=1) as fin:
            # x pair 0 first: it gates the first PE op (sum-of-squares chain)
            xw_early = constp.tile([128, 2 * M], fp8, name="xp0t")
            nc.gpsimd.dma_start(
                out=xw_early.rearrange("p (s m) -> p s m", m=M),
                in_=x_d[0:256, :].rearrange("(s p) m -> p s m", p=128))
            # packed consts: one f32r DMA (ones|identr), one f32 DMA (centn|eye8|bg)
            cr_t = constp.tile([128, 256], f32r)
            nc.gpsimd.dma_start(out=cr_t[:], in_=cr_d[:])
            ones_t = cr_t[:, 0:128]
            identr_t = cr_t[:, 128:256]
            crb_t = constp.tile([128, 128], bf16)
            nc.gpsimd.dma_start(out=crb_t[:], in_=crb_d[:])
            cf_t = constp.tile([128, D + 10], f32)
            nc.gpsimd.dma_start(out=cf_t[:], in_=cf_d[:])
            centn_t = cf_t[:, 0:D]
            identf8_t = cf_t[0:G, D:D + G]
            bg_t = cf_t[0:G, D + G:D + G + 1]
            binp_b = constp.tile([128, EN], f32)
            nc.gpsimd.dma_start(out=binp_b[:], in_=binp_d[:].to_broadcast([128, EN]))

            y_t = [persist.tile([128, M], f32r, name=f"y{e}") for e in range(ET)]
            ybp = [persist.tile([128, 2 * M], fp8, name=f"ybp{c}") for c in range(ET // 2)]
            yT_t = [persist.tile([128, G * BW], f32r, name=f"yT{m}") for m in range(MT)]
            sgc_t = [persist.tile([128, G], f32, name=f"sgc{m}") for m in range(MT)]
            inv_t = persist.tile([128, M], f32)

            # ---------------- phase 1: input, norm, fc_inp, gates, yT ----------------
            with tc.tile_pool(name="xw", bufs=1) as xw, \
                 tc.tile_pool(name="sm1", bufs=1) as sm1, \
                 tc.tile_pool(name="ps1", bufs=1, space="PSUM") as ps1:
                # paired fp8 tiles: contraction pairs along dim1 for DoubleRow.
                # few big DMAs: SP dispatch is ~0.65us each and strictly serial.
                W1W = EN + G + 8  # pad half-block to step%16==0 for DoubleRow
                xp = [xw_early] + [xw.tile([128, 2 * M], fp8, name=f"xp{c}")
                                   for c in range(1, 4)]
                w1p = [xw.tile([128, 2 * W1W], fp8, name=f"w1p{c}") for c in range(4)]
                xpv = [t.rearrange("p (s m) -> p s m", m=M) for t in xp]
                w1v = [t.rearrange("p (s e) -> p s e", e=W1W) for t in w1p]
                for c in range(4):
                    if c > 0:
                        nc.sync.dma_start(
                            out=xpv[c],
                            in_=x_d[c * 256:(c + 1) * 256, :].rearrange(
                                "(s p) m -> p s m", p=128))
                    nc.sync.dma_start(
                        out=w1v[c][:, :, 0:EN + G],
                        in_=w1_d[c * 256:(c + 1) * 256, :].rearrange(
                            "(s p) e -> p s e", p=128))

                # sum of squares over n (partition reduction via ones-matmul)
                ss_ps = ps1.tile([128, M], f32, name="mm_ps", tag="mm_ps", bufs=8)
                for c in range(4):
                    xsq = sm1.tile([128, 2 * M], bf16, name="xsq", bufs=2)
                    nc.scalar.activation(xsq[:], xp[c][:], Act.Square)
                    for s in range(2):
                        nc.tensor.matmul(ss_ps[:], crb_t[:],
                                         xsq[:, s * M:(s + 1) * M],
                                         start=(c == 0 and s == 0),
                                         stop=(c == 3 and s == 1))
                nrm_t = sm1.tile([128, M], f32, name="nrm", bufs=1)
                nc.scalar.activation(nrm_t[:], ss_ps[:], Act.Sqrt, scale=256.0)
                nc.vector.reciprocal(inv_t[:], nrm_t[:])

                DR = mybir.MatmulPerfMode.DoubleRow

                def ychain(es):
                    y_ps = [ps1.tile([128, M], f32, name=f"y_ps{e}", tag="mm_ps",
                                     bufs=8) for e in es]
                    for c in range(4):
                        for k, e in enumerate(es):
                            nc.tensor.matmul(y_ps[k][:],
                                             w1v[c][:, :, e * 128:(e + 1) * 128],
                                             xpv[c], start=(c == 0),
                                             stop=(c == 3), perf_mode=DR)
                    for k, e in enumerate(es):
                        nc.vector.tensor_mul(y_t[e][:], y_ps[k][:], inv_t[:])
                        nc.scalar.activation(ybp[e // 2][:, (e % 2) * M:
                                                         (e % 2 + 1) * M],
                                             y_t[e][:], Act.Copy, scale=32.0)

                def transpose_et(et):
                    g, half = et // 2, et % 2
                    for m in range(MT):
                        t_ps = ps1.tile([128, 128], f32r, name="t_ps",
                                        tag="mm_ps", bufs=8)
                        nc.tensor.transpose(t_ps[:],
                                            y_t[et][:, m * 128:(m + 1) * 128],
                                            identr_t)
                        col = g * BW + half * 128
                        nc.vector.tensor_add(yT_t[m][:, col:col + 128], t_ps[:],
                                             binp_b[:, et * 128:(et + 1) * 128])

                ychain(range(0, 8))
                for m in range(MT):
                    yT3 = yT_t[m].rearrange("p (g c) -> p g c", c=BW)
                    nc.vector.tensor_copy(yT3[:, :, D:D + 2],
                                          cr_t[:, 0:2 * G].rearrange(
                                              "p (g c) -> p g c", c=2))
                # sweepB: 7 chains, transposes of sweepA outputs interleaved
                y_psB = [ps1.tile([128, M], f32, name=f"y_ps{e}", tag="mm_ps",
                                  bufs=8) for e in range(8, 15)]
                for c in range(4):
                    for k, e in enumerate(range(8, 15)):
                        nc.tensor.matmul(y_psB[k][:],
                                         w1v[c][:, :, e * 128:(e + 1) * 128],
                                         xpv[c], start=(c == 0),
                                         stop=(c == 3), perf_mode=DR)
                    transpose_et(2 * c)
                    transpose_et(2 * c + 1)
                for k, e in enumerate(range(8, 15)):
                    nc.vector.tensor_mul(y_t[e][:], y_psB[k][:], inv_t[:])
                    nc.scalar.activation(ybp[e // 2][:, (e % 2) * M:(e % 2 + 1) * M],
                                         y_t[e][:], Act.Copy, scale=32.0)
                ychain([15])
                for et in range(8, ET):
                    transpose_et(et)

                # sigmoid gate logits via DoubleRow (consumed only in phase 3)
                sg_ps = ps1.tile([G, M], f32, name="sg_ps", tag="mm_ps", bufs=8)
                for c in range(4):
                    nc.tensor.matmul(sg_ps[:], w1v[c][:, :, EN:EN + G], xpv[c],
                                     start=(c == 0), stop=(c == 3), perf_mode=DR)
                sgs_t = sm1.tile([G, M], f32, name="sgs", bufs=1)
                nc.vector.tensor_mul(sgs_t[:], sg_ps[:], inv_t[0:G, :])
                nc.scalar.activation(sgs_t[:], sgs_t[:], Act.Sigmoid, bias=bg_t[:])
                # transpose [G, M] -> per m-tile [128, G]
                for m in range(MT):
                    sgc_ps = ps1.tile([128, G], f32, name="sgc_ps", tag="mm_ps", bufs=8)
                    nc.tensor.matmul(sgc_ps[:], sgs_t[:, m * 128:(m + 1) * 128],
                                     identf8_t, start=True, stop=True)
                    nc.vector.tensor_copy(sgc_t[m][:], sgc_ps[:])

            # ---------------- phase 2: gk logits + exp ----------------
            with tc.tile_pool(name="exp2", bufs=1) as exp2, \
                 tc.tile_pool(name="ps2", bufs=1, space="PSUM") as ps2:
                ex_t = [exp2.tile([128, G * KC], f32r, name=f"ex{m}") for m in range(MT)]
                lg_ps = [[ps2.tile([128, 512], f32, name=f"lg{m}_{h}",
                                   tag="lg", bufs=8) for h in range(GH)]
                         for m in range(MT)]
                pairs = {}
                LAG = 2  # in e2 pair-steps
                DR = mybir.MatmulPerfMode.DoubleRow
                E2 = ET // 2
                se_ps_l = [None, None]
                ybv = [t.rearrange("p (s m) -> p s m", m=M) for t in ybp]
                for e2 in range(E2 + LAG):
                    if e2 < E2:
                        w2t = w2p.tile([128, 2 * G * KC], fp8, name="w2t")
                        w2v = w2t.rearrange("p (s j) -> p s j", j=G * KC)
                        nc.sync.dma_start(
                            out=w2v,
                            in_=w2_d[e2 * 256:(e2 + 1) * 256, :].rearrange(
                                "(s p) j -> p s j", p=128))
                        pairs[e2] = w2v
                        for m in range(MT):
                            nc.tensor.matmul(
                                lg_ps[m][0][:],
                                ybv[e2][:, :, m * 128:(m + 1) * 128],
                                pairs[e2][:, :, 0:512],
                                start=(e2 == 0), stop=(e2 == E2 - 1),
                                perf_mode=DR)
                        if e2 == E2 - 1:
                            for m in range(MT):
                                nc.scalar.activation(ex_t[m][:, 0:512],
                                                     lg_ps[m][0][:], Act.Exp,
                                                     scale=1.0 / 256.0)
                    eh = e2 - LAG
                    if eh >= 0:
                        for m in range(MT):
                            nc.tensor.matmul(
                                lg_ps[m][1][:],
                                ybv[eh][:, :, m * 128:(m + 1) * 128],
                                pairs[eh][:, :, 512:1024],
                                start=(eh == 0), stop=(eh == E2 - 1),
                                perf_mode=DR)
                        if eh == E2 - 1:
                            for m in range(MT):
                                nc.scalar.activation(ex_t[m][:, 512:1024],
                                                     lg_ps[m][1][:], Act.Exp,
                                                     scale=1.0 / 256.0)
                    if e2 == E2:
                        se0 = ps2.tile([128, 512], f32, name="se0", tag="lg", bufs=8)
                        se_ps_l[0] = se0
                        for m in range(MT):
                            nc.tensor.matmul(se0[:], ones_t, ex_t[m][:, 0:512],
                                             start=(m == 0), stop=(m == MT - 1))

            # ---------------- phase 3: softmax denom, weights, einsum ----------------
                with tc.tile_pool(name="p3", bufs=1) as p3:
                    ise_t = p3.tile([128, G * KC], f32)
                    nc.vector.reciprocal(ise_t[:, 0:512], se_ps_l[0][:])
                    se1 = ps2.tile([128, 512], f32, name="se1", tag="lg", bufs=8)
                    for m in range(MT):
                        nc.tensor.matmul(se1[:], ones_t, ex_t[m][:, 512:1024],
                                         start=(m == 0), stop=(m == MT - 1))
                    nc.vector.reciprocal(ise_t[:, 512:1024], se1[:])

                    wf_t = [p3.tile([128, G * KC], f32r, name=f"wf{m}") for m in range(MT)]
                    for h in range(GH):
                        for g in range(h * 4, h * 4 + 4):
                            for m in range(MT):
                                sl = slice(g * KC, (g + 1) * KC)
                                nc.vector.scalar_tensor_tensor(
                                    out=wf_t[m][:, sl], in0=ex_t[m][:, sl],
                                    scalar=sgc_t[m][:, g:g + 1], in1=ise_t[:, sl],
                                    op0=Alu.mult, op1=Alu.mult)

                    vd_ps = ps2.tile([128, 512], f32, name="vd_ps", tag="lg", bufs=8)[:, 0:BW]
                    k = 0
                    for g in range(G):
                        for m in range(MT):
                            nc.tensor.matmul(
                                vd_ps[:], wf_t[m][:, g * KC:(g + 1) * KC],
                                yT_t[m][:, g * BW:(g + 1) * BW],
                                start=(k == 0), stop=(k == G * MT - 1))
                            k += 1

                    vlad_t = fin.tile([128, D], f32)
                    nc.vector.scalar_tensor_tensor(
                        out=vlad_t[:], in0=centn_t[:], scalar=vd_ps[:, D:D + 1],
                        in1=vd_ps[:, 0:D], op0=Alu.mult, op1=Alu.add)
                    sq_t = fin.tile([128, D], f32)
                    nc.vector.tensor_mul(sq_t[:], vlad_t[:], vlad_t[:])
                    ss2_t = fin.tile([128, 1], f32)
                    nc.vector.reduce_sum(out=ss2_t[:], in_=sq_t[:],
                                         axis=mybir.AxisListType.X)
                    nr2_t = fin.tile([128, 1], f32)
                    nc.scalar.activation(nr2_t[:], ss2_t[:], Act.Sqrt, scale=128.0)
                    r1_t = fin.tile([128, 1], f32)
                    nc.vector.reciprocal(r1_t[:], nr2_t[:])
                    out_t = fin.tile([128, D], f32)
                    nc.vector.tensor_scalar_mul(out_t[:], vlad_t[:], r1_t[:])
                    nc.sync.dma_start(out=out_d[:], in_=out_t[:])

    nc.compile()
    return nc


def _get_nc():
    if "nc" not in _cache:
        _cache["nc"] = _build_nc()
    return _cache["nc"]


def kernel(x, W_inp, b_inp, W_g, b_g, W_gk, b_gk, centroids):
    from concourse.bass_utils import run_bass_kernel_spmd

    nc = _get_nc()

    x = np.asarray(x, dtype=np.float32)
    X = x.reshape(8, 8, N, 64).transpose(0, 2, 1, 3).reshape(8, N, M)
    import ml_dtypes as _mld
    WgT = ((np.asarray(W_g, np.float64) @ np.asarray(W_inp, np.float64)).T
           ).astype(np.float32)
    W1 = np.ascontiguousarray(
        (np.concatenate([np.asarray(W_inp, np.float32).T, WgT],
                        axis=1) * 16.0).astype(_mld.float8_e4m3))
    import ml_dtypes
    W2 = np.ascontiguousarray((np.asarray(W_gk, np.float32).T * 8.0).astype(ml_dtypes.float8_e4m3))
    bg = (np.asarray(b_g, np.float64)
          + np.asarray(W_g, np.float64) @ np.asarray(b_inp, np.float64)
          ).astype(np.float32)
    binp = np.ascontiguousarray(np.asarray(b_inp, np.float32).reshape(1, EN))
    cr = np.concatenate([np.ones((128, 128), np.float32),
                         np.eye(128, dtype=np.float32)], axis=1)
    cf = np.zeros((128, D + 10), np.float32)
    cf[:, 0:D] = -np.asarray(centroids, np.float32)
    cf[0:G, D:D + G] = np.eye(G, dtype=np.float32)
    cf[0:G, D + G] = bg

    in_maps = []
    for b in range(8):
        in_maps.append({
            "x": np.ascontiguousarray((X[b] * 8.0).astype(_mld.float8_e4m3)), "w1": W1, "w2": W2,
            "binp": binp, "cr": cr, "cf": cf, "crb": np.ones((128, 128), _mld.bfloat16),
        })

    trace = os.environ.get("KERNEL_TRACE") == "1"
    r = run_bass_kernel_spmd(nc, in_maps, core_ids=list(range(8)), trace=trace)
    _cache["last_results"] = r
    return np.stack([r.results[b]["out"].reshape(KC * D) for b in range(8)]).astype(np.float32)

